# revision 13
# baseline (speedup 1.0000x reference)
"""Trainium2 Bass kernel for nn_Depth_CA (depth-coded-aperture Wiener pipeline).

Strategy
--------
Every fft/ifft+shift combo in the reference is a constant 256x256 complex
matrix sandwich Y = A @ X @ A.T.  On the PE array each sandwich is two
matmul groups with the DATA as the stationary operand and host-precomputed
constants [ATr|ATi], [-ATi|ATr] as 512-wide moving operands; PSUM
accumulation implements the complex arithmetic:

    MM1: PSUM = X^T @ A^T   (= (A X)^T)          X      stationary
    MM2: PSUM = (A X) @ A^T (= A X A^T)          (AX)^T stationary

Additional algebraic collapses on top of the baseline:

1. PHASE1 = coef_dc*(x^2+y^2) and PHASE2 = k_c*(fx^2+fy^2) are SEPARABLE
   chirps (q q^T, w w^T), so the whole two-FFT PSF synthesis collapses to
   ONE sandwich  vu3 = C_dc @ M @ C_dc^T  with
       C_dc = A2 diag(w_c) A1 diag(q_dc)   (host-precomputed, 48 units)
       M    = RAD * CA                     (runtime, from H)
   Stage 1 per unit: 32 MMs + 2 complex-mults -> 12 MMs, no cmuls.
2. psf_ifr = conj(psffr)/N^2 (real psf), so the Gc psf Gc sandwich for the
   Wiener numerator disappears; the 1/N^2 is a global scale that cancels
   in the final recov/max(recov).  The conj rides on a sign-flip in the
   Wiener complex multiply.
3. psf normalization folds into the psffr PSUM drain (scale-by-1/sum on
   the drain copy), and the mid-pipeline result/max(result) provably
   cancels and is skipped.
4. The blur/Wiener inverse transforms are mathematically real, so their
   second matmul groups compute only the real part.

Sharding: depths padded 15->16, 2 per core across 8 cores.  Each core
computes its 6 (depth, band) PSF units, the 12 image/result FFTs
(replicated), the blur and Wiener stages for its own depths, with
per-batch AllReduce(add) collectives for the depth-summed `result` and
one AllReduce(max) for the final normalization.
"""
import os
import sys

for _p in ("/opt/trn_rl_repo", os.path.expanduser("~/.axon_site/_ro/trn_rl_repo")):
    if os.path.isdir(_p) and _p not in sys.path:
        sys.path.insert(0, _p)

import numpy as np

N = 256
ND, NB, B = 15, 3, 4
NDP = 16               # padded depth count
NCORES = 8
DPC = NDP // NCORES    # depths per core = 2

# ---------------------------------------------------------------- host constants
def _host_constants():
    ZI, Z0, RADII, PX = 0.05, 2.5, 0.002, 6.22e-6
    F_ = 1.0 / (1.0 / ZI + 1.0 / Z0)
    L_SEN = PX * N
    L_LEN = 2 * RADII * 2
    LAMB = np.array([460.0, 550.0, 640.0]) * 1e-9

    def deta(l_um):
        l = np.asarray(l_um, dtype=np.float64)
        return (1.5375 + 0.00829045 * l**-2 - 0.000211046 * l**-4) - 1.0

    R_ = F_ * deta(5.5e-7 * 1e6)
    FLMB = R_ / deta(LAMB * 1e6)
    ZS = np.sort(-3 * np.log(np.linspace(0.9, 11, ND)) + 8)
    DU = L_LEN / N
    u = np.arange(-L_LEN / 2, L_LEN / 2, DU)
    RAD = (np.sqrt(u[None, :] ** 2 + u[:, None] ** 2) <= RADII).astype(np.float64)
    fx1 = np.fft.fftshift(np.arange(-1 / (2 * DU), 1 / (2 * DU), 1 / L_LEN))

    K_ = 2 * np.pi / LAMB
    COEF = (-K_ / (2 * FLMB[0]))[None, :] + K_[None, :] / (2 * ZS[:, None]) \
        + (np.pi * (L_LEN - L_SEN) / (LAMB * ZI * L_LEN))[None, :]
    # separable chirps
    q1 = np.exp(1j * COEF[:, :, None] * (u ** 2)[None, None, :])           # (15,3,N)
    w2 = np.exp(-1j * (np.pi * LAMB * ZI * L_LEN / L_SEN)[:, None]
                * (fx1 ** 2)[None, :])                                     # (3,N)

    j = np.arange(N)
    F = np.exp(-2j * np.pi * np.outer(j, j) / N)
    G = np.conj(F) / N
    P = np.zeros((N, N))
    P[j, (j + N // 2) % N] = 1.0
    A1 = F @ P
    A2 = P @ G
    Fc = P @ F @ P
    Gc = P @ G @ P
    # collapsed per-unit constants C_dc = A2 diag(w_c) A1 diag(q_dc)
    C = np.empty((ND, NB, N, N), np.complex128)
    for d in range(ND):
        for c in range(NB):
            C[d, c] = (A2 * w2[c][None, :]) @ (A1 * q1[d, c][None, :])
    return C, RAD, Fc, Gc


def _pack_moving(A):
    """constant A -> float32 [2 variants, 2 k-chunks, 128, 512] moving ops."""
    AT = A.T.copy()
    out = np.empty((2, 2, 128, 512), np.float32)
    for k in range(2):
        r = AT.real[k * 128:(k + 1) * 128, :]
        i = AT.imag[k * 128:(k + 1) * 128, :]
        out[0, k, :, 0:256] = r
        out[0, k, :, 256:512] = i
        out[1, k, :, 0:256] = -i
        out[1, k, :, 256:512] = r
    return out


def _pack_flat(A):
    """-> [128, 2048] with col = (v*2+k)*512 + n."""
    return _pack_moving(A).reshape(4, 128, 512).transpose(1, 0, 2).reshape(128, 2048)


_CONST_CACHE = {}


def _get_device_arrays():
    """Host constants packed into the device DMA layouts."""
    if "dev" not in _CONST_CACHE:
        C, RAD, Fc, Gc = _host_constants()
        movFG = np.concatenate([_pack_flat(Fc), _pack_flat(Gc)], axis=1)  # [128,4096]
        crows = []
        for d in range(NDP):
            dd = d if d < ND else 0
            for c in range(NB):
                crows.append(_pack_flat(C[dd, c]))
        call = np.stack(crows)                                            # [48,128,2048]
        radp = np.empty((128, 512), np.float32)
        for k in range(2):
            radp[:, k * 256:(k + 1) * 256] = RAD[k * 128:(k + 1) * 128, :]
        R = np.kron(np.eye(16), np.ones((1, 16))).astype(np.float32)
        _CONST_CACHE["dev"] = (np.ascontiguousarray(movFG), np.ascontiguousarray(call),
                               np.ascontiguousarray(radp), R)
    return _CONST_CACHE["dev"]


# ---------------------------------------------------------------- device program
_REPS = int(os.environ.get("BASS_KERNEL_REPS", "1"))

FC_I, GC_I = 0, 1


def _build_program():
    host_arrays = _get_device_arrays()
    reps = _REPS
    import concourse.bass as bass
    import concourse.bass_isa as bass_isa
    import concourse.bacc as bacc
    import concourse.mybir as mybir
    import concourse.tile as tile

    dt = mybir.dt
    ALU = mybir.AluOpType
    ACTF = mybir.ActivationFunctionType

    movFG_h, call_h, radp_h, R_h = host_arrays

    nc = bacc.Bacc("TRN2", target_bir_lowering=False, debug=False,
                   num_devices=NCORES)

    def inline(data, name, f32r=False):
        h = nc.inline_tensor(np.ascontiguousarray(data), name=name)
        if f32r:
            mls = nc.lookup_mls(h)
            mls.dtype = dt.float32r
            h = bass.DRamTensorHandle(name, list(data.shape), dt.float32r)
        return h.ap()

    movFG_d = inline(movFG_h, "movfg", f32r=True)              # [128, 4096]
    call_d = inline(call_h, "call", f32r=True)                 # [48, 128, 2048]
    radp_d = inline(radp_h, "radp")                            # [128, 512]
    r_d = inline(R_h, "rmat")                                  # [16, 256]

    img_d = nc.dram_tensor("imgf", [128, 6144], dt.float32r, kind="ExternalInput").ap()
    map_d = nc.dram_tensor("mapf", [B, 128, DPC * 512], dt.float32, kind="ExternalInput").ap()
    ht_d = nc.dram_tensor("ht", [16, 16], dt.float32, kind="ExternalInput").ap()
    par_d = nc.dram_tensor("param", [1, 1], dt.float32, kind="ExternalInput").ap()
    mask_d = nc.dram_tensor("mask", [1, DPC], dt.float32, kind="ExternalInput").ap()
    out_d = nc.dram_tensor("out_recov", [DPC, NB, B, 128, 512], dt.float32, kind="ExternalOutput").ap()

    with tile.TileContext(nc) as tc:
        with (
            tc.tile_pool(name="res", bufs=1) as res,
            tc.tile_pool(name="wk", bufs=2) as wk,
            tc.tile_pool(name="ps", bufs=4, space="PSUM") as ps,
            tc.tile_pool(name="dram", bufs=1, space="DRAM") as dram,
        ):
            # ---------------- resident constants (single DMAs)
            movall = res.tile([128, 4096], dt.float32r, tag="movall", name="movall")
            nc.sync.dma_start(movall[:], movFG_d[:])

            def mov(a, v, k):
                o = ((a * 2 + v) * 2 + k) * 512
                return movall[:, o:o + 512]

            radt = res.tile([128, 512], dt.float32, tag="radt", name="radt")
            nc.sync.dma_start(radt[:], radp_d[:])

            par1 = res.tile([1, 1], dt.float32, tag="par1", name="par1")
            nc.sync.dma_start(par1[:], par_d[:])
            par128 = res.tile([128, 1], dt.float32, tag="par128", name="par128")
            nc.gpsimd.partition_broadcast(par128[:], par1[:])
            mask1 = res.tile([1, DPC], dt.float32, tag="mask1", name="mask1")
            nc.sync.dma_start(mask1[:], mask_d[:])
            mask128 = res.tile([128, DPC], dt.float32, tag="mask128", name="mask128")
            nc.gpsimd.partition_broadcast(mask128[:], mask1[:])

            # ---------------- CA = R^T @ (H @ R); M = RAD * CA  (stationary f32r)
            ht_t = res.tile([16, 16], dt.float32, tag="ht_t", name="ht_t")
            r_t = res.tile([16, 256], dt.float32, tag="r_t", name="r_t")
            nc.sync.dma_start(ht_t[:], ht_d[:])
            nc.sync.dma_start(r_t[:], r_d[:])
            ca_mid_ps = ps.tile([16, 256], dt.float32, tag="psB", bufs=4, name="ca_mid_ps")
            nc.tensor.matmul(ca_mid_ps[:], ht_t[:], r_t[:], start=True, stop=True)
            ca_mid = res.tile([16, 256], dt.float32, tag="ca_mid", name="ca_mid")
            nc.vector.tensor_copy(ca_mid[:], ca_mid_ps[:])
            m_stat = [res.tile([128, 256], dt.float32r, tag=f"mst{mb}", name=f"mst{mb}")
                      for mb in range(2)]
            for mb in range(2):
                ca_ps = ps.tile([128, 256], dt.float32, tag="psB", bufs=4, name=f"ca_ps{mb}")
                nc.tensor.matmul(ca_ps[:], r_t[:, mb * 128:(mb + 1) * 128],
                                 ca_mid[:], start=True, stop=True)
                nc.vector.tensor_tensor(m_stat[mb][:], ca_ps[:],
                                        radt[:, mb * 256:(mb + 1) * 256], op=ALU.mult)

            # ---------------- helpers
            MM1_NAMES = ("s1a", "pfa", "ifa", "bla", "rfa", "wna")

            def mm_sandwich_half(stat, movfn, is_complex, name):
                """PSUM[mb] = S^T @ A^T.  `stat` = list of 2 per-k-chunk APs:
                complex: [128,512] ([Re | Im]); real: [128,256]."""
                ptag = "psA" if name in MM1_NAMES else "psB"
                psums = []
                for mb in range(2):
                    acc = ps.tile([128, 512], dt.float32, tag=ptag, bufs=4, name=f"{name}_ps{mb}")
                    mms = []
                    for k in range(2):
                        mms.append((stat[k][:, mb * 128:(mb + 1) * 128], movfn(0, k)))
                        if is_complex:
                            mms.append((stat[k][:, 256 + mb * 128:256 + (mb + 1) * 128],
                                        movfn(1, k)))
                    for i, (lhsT, rhs) in enumerate(mms):
                        nc.tensor.matmul(acc[:], lhsT, rhs,
                                         start=(i == 0), stop=(i == len(mms) - 1))
                    psums.append(acc)
                return psums

            def mm_sandwich_real_out(stat, movfn, name):
                """Re-only PSUM[mb][128,256] = Re(S^T @ A^T), S complex packed."""
                psums = []
                for mb in range(2):
                    acc = ps.tile([128, 256], dt.float32, tag="psB", bufs=4, name=f"{name}_ps{mb}")
                    mms = []
                    for k in range(2):
                        mms.append((stat[k][:, mb * 128:(mb + 1) * 128],
                                    movfn(0, k)[:, 0:256]))
                        mms.append((stat[k][:, 256 + mb * 128:256 + (mb + 1) * 128],
                                    movfn(1, k)[:, 0:256]))
                    for i, (lhsT, rhs) in enumerate(mms):
                        nc.tensor.matmul(acc[:], lhsT, rhs,
                                         start=(i == 0), stop=(i == len(mms) - 1))
                    psums.append(acc)
                return psums

            def drain_f32r(psums, name):
                """PSUM -> SBUF f32r bridge.  blur/wiener drains go scalar-only
                to keep the (bottleneck) vector queue free."""
                scalar_only = name in ("blu", "wnu")
                dtag, dbufs = ("drB", 6) if scalar_only else ("drA", 6)
                out = [wk.tile([128, 512], dt.float32r, tag=dtag, bufs=dbufs, name=f"{name}{mb}")
                       for mb in range(2)]
                nc.scalar.copy(out[0][:], psums[0][:])
                if scalar_only:
                    nc.scalar.copy(out[1][:], psums[1][:])
                else:
                    nc.vector.tensor_copy(out[1][:], psums[1][:])
                return out

            CMS_BUFS = 16

            def cmul(out_rb, x_rb, y_rb, conj_x=False):
                """one-rb complex mult: out [128,512] = x * y ([Re|Im] packed).
                conj_x: multiply conj(x) * y instead."""
                xr, xi = x_rb[:, 0:256], x_rb[:, 256:512]
                yr, yi = y_rb[:, 0:256], y_rb[:, 256:512]
                t1 = wk.tile([128, 256], dt.float32, tag="cms", bufs=CMS_BUFS, name="cmt1")
                t2 = wk.tile([128, 256], dt.float32, tag="cms", bufs=CMS_BUFS, name="cmt2")
                t3 = wk.tile([128, 256], dt.float32, tag="cms", bufs=CMS_BUFS, name="cmt3")
                t4 = wk.tile([128, 256], dt.float32, tag="cms", bufs=CMS_BUFS, name="cmt4")
                nc.vector.tensor_tensor(t1[:], xr, yr, op=ALU.mult)
                nc.gpsimd.tensor_tensor(t2[:], xi, yi, op=ALU.mult)
                nc.vector.tensor_tensor(out_rb[:, 0:256], t1[:], t2[:],
                                        op=(ALU.add if conj_x else ALU.subtract))
                nc.vector.tensor_tensor(t3[:], xr, yi, op=ALU.mult)
                nc.gpsimd.tensor_tensor(t4[:], xi, yr, op=ALU.mult)
                nc.vector.tensor_tensor(out_rb[:, 256:512], t3[:], t4[:],
                                        op=(ALU.subtract if conj_x else ALU.add))

            # ---------------- resident per-unit products
            psffr_t = [res.tile([128, 512], dt.float32, tag=f"psffr{i}", name=f"psffr{i}")
                       for i in range(DPC * NB * 2)]

            imgft_dr = dram.tile([B * NB, 128, 1024], dt.float32, name="imgft_dr")
            mag2_dr = dram.tile([DPC * NB * B, 128, 512], dt.float32, name="mag2_dr")
            kker_dr = dram.tile([DPC * NB, 128, 1024], dt.float32, name="kker_dr")

            pid6 = nc.gpsimd.partition_id() * (DPC * NB)

            def emit_imgft(f):
                imS = wk.tile([128, 512], dt.float32r, tag="imS", bufs=3, name="imS")
                nc.sync.dma_start(imS[:], img_d[:, f * 512:(f + 1) * 512])
                stat = [imS[:, 0:256], imS[:, 256:512]]
                iu1 = drain_f32r(mm_sandwich_half(
                    stat, lambda v, k: mov(FC_I, v, k), False, "ifa"), "ifu")
                ip2 = mm_sandwich_half(iu1, lambda v, k: mov(FC_I, v, k), True, "ifb")
                imo = wk.tile([128, 1024], dt.float32, tag="cfld", bufs=3, name="imo")
                nc.scalar.copy(imo[:, 0:512], ip2[0][:])
                nc.vector.tensor_copy(imo[:, 512:1024], ip2[1][:])
                nc.scalar.dma_start(imgft_dr[f], imo[:])

            for _rep in range(reps):
                cc_in = [dram.tile([NB, 128, 512], dt.bfloat16, name=f"cc_in{b}_r{_rep}")
                         for b in range(B)]
                cc_out = [dram.tile([NB, 128, 512], dt.bfloat16, name=f"cc_out{b}_r{_rep}",
                                    addr_space="Shared") for b in range(B)]
                ccm_in = dram.tile([1, 16], dt.float32, name=f"ccm_in_r{_rep}")
                ccm_out = dram.tile([1, 16], dt.float32, name=f"ccm_out_r{_rep}", addr_space="Shared")

                # ---- stage-1 unit emitter: psf, psffr, K for unit u
                def emit_unit(u):
                    cu = wk.tile([128, 2048], dt.float32r, tag="cu", bufs=2, name="cu")
                    nc.gpsimd.dma_start(cu[:], call_d[bass.ds(pid6 + u, 1)])

                    def cmov(v, k):
                        o = (v * 2 + k) * 512
                        return cu[:, o:o + 512]

                    # vu3 = C M C^T : one collapsed sandwich
                    u1 = drain_f32r(mm_sandwich_half(
                        [m_stat[0][:], m_stat[1][:]], cmov, False, "s1a"), "s1u1")
                    ps2 = mm_sandwich_half(u1, cmov, True, "s1b")
                    # psf (unnormalized) = |vu3|^2, rb-packed [128,512], f32r
                    psfu = wk.tile([128, 512], dt.float32r, tag="psfu", bufs=2, name="psfu")
                    for rb in range(2):
                        t1 = wk.tile([128, 256], dt.float32, tag="cms", bufs=CMS_BUFS, name="sq1")
                        t2 = wk.tile([128, 256], dt.float32, tag="cms", bufs=CMS_BUFS, name="sq2")
                        nc.scalar.activation(t1[:], ps2[rb][:, 0:256], ACTF.Square)
                        nc.scalar.activation(t2[:], ps2[rb][:, 256:512], ACTF.Square)
                        nc.vector.tensor_tensor(psfu[:, rb * 256:(rb + 1) * 256],
                                                t1[:], t2[:], op=ALU.add)
                    sums = wk.tile([128, 1], dt.float32, tag="sums", name="sums")
                    nc.vector.tensor_reduce(sums[:], psfu[:], axis=mybir.AxisListType.X, op=ALU.add)
                    tot128 = wk.tile([128, 1], dt.float32, tag="tot128", name="tot128")
                    nc.gpsimd.partition_all_reduce(tot128[:], sums[:], channels=128,
                                                   reduce_op=bass_isa.ReduceOp.add)
                    inv128 = wk.tile([128, 1], dt.float32, tag="inv128", name="inv128")
                    nc.vector.reciprocal(inv128[:], tot128[:])
                    # psffr = Fc psf Fc / sum  (normalization folded into drain)
                    psts = [psfu[:, 0:256], psfu[:, 256:512]]
                    pu1 = drain_f32r(mm_sandwich_half(
                        psts, lambda v, k: mov(FC_I, v, k), False, "pfa"), "pfu")
                    pp2 = mm_sandwich_half(pu1, lambda v, k: mov(FC_I, v, k), True, "pfb")
                    nc.scalar.activation(psffr_t[u * 2 + 0][:], pp2[0][:],
                                         ACTF.Copy, scale=inv128[:])
                    nc.vector.tensor_scalar_mul(psffr_t[u * 2 + 1][:], pp2[1][:], inv128[:])
                    # K = conj(psffr)/(|psffr|^2 + param); conj deferred to the
                    # Wiener cmul, global 1/N^2 cancels in final normalize
                    kk = wk.tile([128, 1024], dt.float32, tag="cfld", bufs=3, name="kk")
                    for rb in range(2):
                        fr = psffr_t[u * 2 + rb][:, 0:256]
                        fi = psffr_t[u * 2 + rb][:, 256:512]
                        t1 = wk.tile([128, 256], dt.float32, tag="cms", bufs=CMS_BUFS, name="ab1")
                        t2 = wk.tile([128, 256], dt.float32, tag="cms", bufs=CMS_BUFS, name="ab2")
                        nc.vector.tensor_tensor(t1[:], fr, fr, op=ALU.mult)
                        nc.gpsimd.tensor_tensor(t2[:], fi, fi, op=ALU.mult)
                        # (fr^2 + par) + fi^2 fused in one op
                        nc.vector.scalar_tensor_tensor(t1[:], t1[:], par128[:], t2[:],
                                                       op0=ALU.add, op1=ALU.add)
                        invp = wk.tile([128, 256], dt.float32, tag="cms", bufs=CMS_BUFS, name="invp")
                        nc.vector.reciprocal(invp[:], t1[:])
                        nc.vector.tensor_tensor(kk[:, rb * 512:rb * 512 + 256],
                                                fr, invp[:], op=ALU.mult)
                        nc.gpsimd.tensor_tensor(kk[:, rb * 512 + 256:rb * 512 + 512],
                                                fi, invp[:], op=ALU.mult)
                    nc.scalar.dma_start(kker_dr[u], kk[:])

                # ---- blur emitter for one (b, c) field
                def emit_blur(b, c, mapt):
                    f = b * NB + c
                    imf = wk.tile([128, 1024], dt.float32, tag="imf", bufs=3, name="imf")
                    nc.scalar.dma_start(imf[:], imgft_dr[f])
                    racc = wk.tile([128, 512], dt.float32, tag="racc", bufs=4, name="racc")
                    nc.gpsimd.memset(racc[:], 0.0)
                    for dl in range(DPC):
                        u = dl * NB + c
                        bp = wk.tile([128, 1024], dt.float32r, tag="cprod", bufs=3, name="bp")
                        for rb in range(2):
                            cmul(bp[:, rb * 512:(rb + 1) * 512],
                                 imf[:, rb * 512:(rb + 1) * 512], psffr_t[u * 2 + rb])
                        bps = [bp[:, 0:512], bp[:, 512:1024]]
                        bu1 = drain_f32r(mm_sandwich_half(
                            bps, lambda v, k: mov(GC_I, v, k), True, "bla"), "blu")
                        bp2 = mm_sandwich_real_out(
                            bu1, lambda v, k: mov(GC_I, v, k), "blb")
                        # blur of a positive image by a positive psf is
                        # positive, so |Re(.)| == Re(.) and the reference Abs
                        # is a no-op: multiply PSUM by Map directly.
                        for rb in range(2):
                            t2 = wk.tile([128, 256], dt.float32, tag="cms", bufs=CMS_BUFS, name="bm2")
                            nc.vector.tensor_tensor(
                                t2[:], bp2[rb][:], mapt[:, (dl * 2 + rb) * 256:(dl * 2 + rb + 1) * 256],
                                op=ALU.mult)
                            nc.gpsimd.tensor_tensor(racc[:, rb * 256:(rb + 1) * 256],
                                                    racc[:, rb * 256:(rb + 1) * 256],
                                                    t2[:], op=ALU.add)
                    racc16 = wk.tile([128, 512], dt.bfloat16, tag="racc16", bufs=4, name="racc16")
                    nc.scalar.copy(racc16[:], racc[:])
                    nc.sync.dma_start(cc_in[b][c], racc16[:])

                def load_mapt(b):
                    mapt = wk.tile([128, DPC * 512], dt.float32, tag="mapt", bufs=2, name="mapt")
                    nc.sync.dma_start(mapt[:], map_d[b])
                    return mapt

                def emit_cc(b):
                    nc.gpsimd.collective_compute(
                        "AllReduce", ALU.add,
                        replica_groups=[list(range(NCORES))],
                        ins=[cc_in[b][:]], outs=[cc_out[b][:]],
                    )

                unitmax = res.tile([128, DPC * NB * B], dt.float32, tag="unitmax",
                                   name=f"unitmax_r{_rep}")

                # ---- wiener emitter for one batch b
                def emit_wiener(b):
                    for c in range(NB):
                        rres = wk.tile([128, 512], dt.bfloat16, tag="rres", bufs=2, name="rres")
                        nc.sync.dma_start(rres[:], cc_out[b][c])
                        res_t = wk.tile([128, 512], dt.float32r, tag="res_t", bufs=2, name="res_t")
                        nc.vector.tensor_copy(res_t[:], rres[:])
                        rsts = [res_t[:, 0:256], res_t[:, 256:512]]
                        ru1 = drain_f32r(mm_sandwich_half(
                            rsts, lambda v, k: mov(FC_I, v, k), False, "rfa"), "rfu")
                        rp2 = mm_sandwich_half(ru1, lambda v, k: mov(FC_I, v, k), True, "rfb")
                        resfr = wk.tile([128, 1024], dt.float32, tag="cfld", bufs=3, name="resfr")
                        nc.scalar.copy(resfr[:, 0:512], rp2[0][:])
                        nc.vector.tensor_copy(resfr[:, 512:1024], rp2[1][:])
                        for dl in range(DPC):
                            u = dl * NB + c
                            kkt = wk.tile([128, 1024], dt.float32, tag="kkt", bufs=3, name="kkt")
                            nc.scalar.dma_start(kkt[:], kker_dr[u])
                            wn = wk.tile([128, 1024], dt.float32r, tag="cprod", bufs=3, name="wn")
                            for rb in range(2):
                                cmul(wn[:, rb * 512:(rb + 1) * 512],
                                     kkt[:, rb * 512:(rb + 1) * 512],
                                     resfr[:, rb * 512:(rb + 1) * 512], conj_x=True)
                            wns = [wn[:, 0:512], wn[:, 512:1024]]
                            wu1 = drain_f32r(mm_sandwich_half(
                                wns, lambda v, k: mov(GC_I, v, k), True, "wna"), "wnu")
                            wp2 = mm_sandwich_real_out(
                                wu1, lambda v, k: mov(GC_I, v, k), "wnb")
                            mi = (dl * NB + c) * B + b
                            mag2 = wk.tile([128, 512], dt.float32, tag="mag2", bufs=2, name="mag2")
                            for rb in range(2):
                                nc.scalar.activation(mag2[:, rb * 256:(rb + 1) * 256],
                                                     wp2[rb][:], ACTF.Abs)
                            nc.vector.tensor_reduce(unitmax[:, mi:mi + 1], mag2[:],
                                                    axis=mybir.AxisListType.X, op=ALU.max)
                            nc.scalar.dma_start(mag2_dr[mi], mag2[:])

                # ---- software-pipelined emission order.  Engines execute
                # their queues in order, so emission order is priority:
                # stage-1 units are reordered [dl0, dl1] per band so blur
                # (b=0, c) can issue as soon as band c's two psffrs exist,
                # pulling the first AllReduce ~80us earlier; wiener(b) lags
                # CC(b) by two slots to avoid head-of-line blocking.
                mapt0 = load_mapt(0)
                fno = 0
                for c in range(NB):
                    for u in (c, NB + c):
                        emit_imgft(fno)
                        emit_imgft(fno + 1)
                        fno += 2
                        emit_unit(u)
                    emit_blur(0, c, mapt0)
                emit_cc(0)
                mapt1 = load_mapt(1)
                for c in range(NB):
                    emit_blur(1, c, mapt1)
                emit_cc(1)
                mapt2 = load_mapt(2)
                for c in range(NB):
                    emit_blur(2, c, mapt2)
                emit_cc(2)
                emit_wiener(0)
                mapt3 = load_mapt(3)
                for c in range(NB):
                    emit_blur(3, c, mapt3)
                emit_cc(3)
                for b in range(1, B):
                    emit_wiener(b)

                # ======== global max + final normalize
                nc.vector.tensor_scalar_mul(unitmax[:, NB * B:2 * NB * B],
                                            unitmax[:, NB * B:2 * NB * B], mask128[:, 1:2])
                mx = wk.tile([128, 1], dt.float32, tag="mx", name="mx")
                nc.vector.tensor_reduce(mx[:], unitmax[:], axis=mybir.AxisListType.X, op=ALU.max)
                gmx128 = wk.tile([128, 1], dt.float32, tag="gmx128", name="gmx128")
                nc.gpsimd.partition_all_reduce(gmx128[:], mx[:], channels=128,
                                               reduce_op=bass_isa.ReduceOp.max)
                ones16 = wk.tile([1, 16], dt.float32, tag="ones16", name="ones16")
                nc.vector.memset(ones16[:], 1.0)
                gmx16 = wk.tile([1, 16], dt.float32, tag="gmx16", name="gmx16")
                nc.vector.tensor_scalar_mul(gmx16[:], ones16[:], gmx128[0:1, :])
                nc.sync.dma_start(ccm_in[:], gmx16[:])
                nc.gpsimd.collective_compute(
                    "AllReduce", ALU.max,
                    replica_groups=[list(range(NCORES))],
                    ins=[ccm_in[:]], outs=[ccm_out[:]],
                )
                gm = wk.tile([1, 1], dt.float32, tag="gm", name="gm")
                nc.sync.dma_start(gm[:], ccm_out[0:1, 0:1])
                ginv = wk.tile([1, 1], dt.float32, tag="ginv", name="ginv")
                nc.vector.reciprocal(ginv[:], gm[:])
                ginv128 = wk.tile([128, 1], dt.float32, tag="ginv128", name="ginv128")
                nc.gpsimd.partition_broadcast(ginv128[:], ginv[:])

                for dl in range(DPC):
                    for c in range(NB):
                        for b in range(B):
                            mi = (dl * NB + c) * B + b
                            m2 = wk.tile([128, 512], dt.float32, tag="finm", bufs=4, name="m2")
                            nc.sync.dma_start(m2[:], mag2_dr[mi])
                            o = wk.tile([128, 512], dt.float32, tag="fino", bufs=4, name="o")
                            if mi % 2 == 0:
                                nc.scalar.activation(o[:], m2[:], ACTF.Copy, scale=ginv128[:])
                                nc.scalar.dma_start(out_d[dl, c, b], o[:])
                            else:
                                nc.vector.tensor_scalar_mul(o[:], m2[:], ginv128[:])
                                nc.sync.dma_start(out_d[dl, c, b], o[:])

    nc.compile()
    return nc


_PROG_CACHE = {}


def _get_program():
    if "nc" not in _PROG_CACHE:
        _PROG_CACHE["nc"] = _build_program()
    return _PROG_CACHE["nc"]


# ---------------------------------------------------------------- cached runner
def _make_runner():
    """Build the jitted SPMD callable once; reuse across kernel() calls."""
    import jax
    from jax.sharding import Mesh, PartitionSpec
    from jax.experimental.shard_map import shard_map
    import concourse.mybir as mybir
    from concourse import bass2jax

    bass2jax.install_neuronx_cc_hook()
    nc = _get_program()

    partition_name = nc.partition_id_tensor.name if nc.partition_id_tensor else None
    in_names, out_names, out_avals, zero_shapes = [], [], [], []
    for alloc in nc.m.functions[0].allocations:
        if not isinstance(alloc, mybir.MemoryLocationSet):
            continue
        if not alloc.memorylocations:
            continue
        name = alloc.memorylocations[0].name
        if alloc.kind == "ExternalInput":
            if name != partition_name:
                in_names.append(name)
        elif alloc.kind == "ExternalOutput":
            out_names.append(name)
            shape = tuple(alloc.tensor_shape)
            dtype = mybir.dt.np(alloc.dtype)
            out_avals.append(jax.core.ShapedArray(shape, dtype))
            zero_shapes.append((shape, dtype))
    n_params = len(in_names)
    n_outs = len(out_avals)
    all_in_names = list(in_names) + list(out_names)
    if partition_name is not None:
        all_in_names.append(partition_name)
    donate = tuple(range(n_params, n_params + n_outs))

    def _body(*args):
        operands = list(args)
        if partition_name is not None:
            operands.append(bass2jax.partition_id_tensor())
        outs = bass2jax._bass_exec_p.bind(
            *operands,
            out_avals=tuple(out_avals),
            in_names=tuple(all_in_names),
            out_names=tuple(out_names),
            lowering_input_output_aliases=(),
            sim_require_finite=True,
            sim_require_nnan=True,
            nc=nc,
        )
        return tuple(outs)

    devices = jax.devices()[:NCORES]
    mesh = Mesh(np.asarray(devices), ("core",))
    in_specs = (PartitionSpec("core"),) * (n_params + n_outs)
    out_specs = (PartitionSpec("core"),) * n_outs
    sharded = jax.jit(
        shard_map(_body, mesh=mesh, in_specs=in_specs, out_specs=out_specs,
                  check_rep=False),
        donate_argnums=donate, keep_unused=True)

    def run(in_maps):
        concat_in = [
            np.concatenate([np.asarray(m[name]) for m in in_maps], axis=0)
            for name in in_names
        ]
        concat_zeros = [
            np.zeros((NCORES * s[0], *s[1:]), d) for (s, d) in zero_shapes
        ]
        out_arrs = sharded(*concat_in, *concat_zeros)
        return [
            {name: np.asarray(out_arrs[i]).reshape(NCORES, *out_avals[i].shape)[c]
             for i, name in enumerate(out_names)}
            for c in range(NCORES)
        ]

    return run


def _get_runner():
    if "run" not in _PROG_CACHE:
        _PROG_CACHE["run"] = _make_runner()
    return _PROG_CACHE["run"]


# ---------------------------------------------------------------- entry point
def _build_in_maps(img, Map, H, parameter):
    # img fields (b,c) -> [128, 6144]: col = (f*2+k)*256 + x
    imgt = img.transpose(0, 3, 1, 2).reshape(B * NB, 2, 128, 256)
    imgf = np.ascontiguousarray(imgt.transpose(2, 0, 1, 3).reshape(128, B * NB * 512))
    # Map -> per-core [4, 128, DPC*512]: col = (dl*2+rb)*256 + x
    mapt = Map.transpose(3, 0, 1, 2).reshape(ND, B, 2, 128, 256)
    ht = np.ascontiguousarray(H.reshape(16, 16).T)
    par = parameter.reshape(1, 1)
    in_maps = []
    for core in range(NCORES):
        mp = np.zeros((B, 128, DPC * 512), np.float32)
        msk = np.zeros((1, DPC), np.float32)
        for dl in range(DPC):
            d = core * DPC + dl
            if d < ND:
                fld = mapt[d].transpose(0, 2, 1, 3).reshape(B, 128, 512)
                mp[:, :, dl * 512:(dl + 1) * 512] = fld
                msk[0, dl] = 1.0
        in_maps.append({
            "imgf": imgf, "mapf": mp, "ht": ht, "param": par, "mask": msk,
        })
    return in_maps


def kernel(img, Map, H, parameter):
    img = np.ascontiguousarray(np.asarray(img, np.float32))
    Map = np.ascontiguousarray(np.asarray(Map, np.float32))
    H = np.asarray(H, np.float32)
    parameter = np.asarray(parameter, np.float32)

    try:
        run = _get_runner()
    except Exception:
        run = None

    in_maps = _build_in_maps(img, Map, H, parameter)

    if run is not None:
        try:
            results = run(in_maps)
        except Exception:
            run = None
    if run is None:
        from concourse.bass_utils import run_bass_kernel_spmd
        rr = run_bass_kernel_spmd(_get_program(), in_maps,
                                  core_ids=list(range(NCORES)))
        results = rr.results

    out = np.empty((B, 256, 256, NB * ND), np.float32)
    for core in range(NCORES):
        rec = results[core]["out_recov"]            # [DPC, NB, B, 256, 256]
        for dl in range(DPC):
            d = core * DPC + dl
            if d >= ND:
                continue
            for c in range(NB):
                for b in range(B):
                    out[b, :, :, c * ND + d] = (
                        rec[dl, c, b].reshape(128, 2, 256)
                        .transpose(1, 0, 2).reshape(256, 256))
    return out
